# revision 1
# baseline (speedup 1.0000x reference)
"""GCN encoder (3x GCNConv sharing one normalized adjacency) on 8 TRN2 NeuronCores.

Strategy:
  - Fold the symmetric GCN norm  norm(r,c) = dis[r]*dis[c]  into per-node
    scales: pre-scale rows by dis (host for x, epilogue for h), post-scale
    aggregates by dis[c]. Per-edge messages then need no per-edge weights.
  - Shard destination nodes across the 8 cores (6272 nodes/core after
    padding N=50000 -> 50176). Edges live on the core that owns their
    destination (edge-cut partitioning per the sharding hint).
  - Per conv: gather source rows with dma_gather (fp16 rows, 256B), build
    {0,1} one-hot matrices on the vector engine (is_equal vs an iota), and
    scatter-add via TensorE matmuls accumulating in PSUM per 128-dst tile.
  - Node features are republished between convs with AllGather collectives.
  - mu and logstd share one pass: Wc = [W_mu | W_logstd] (both 64 wide).
"""

import numpy as np

N = 50000
E = 800000
IN = 128
HID = 128
OUT = 64
NCORES = 8
SH = 6272                 # nodes per core (padded)
NPAD = SH * NCORES        # 50176
NT = SH // 128            # 49 dst tiles per core
LO = 32768                # rows in the "lo" gather table (int16 limit)
HIR = NPAD - LO           # rows in the "hi" gather table
TB = 6                    # dst tiles per gather batch
OHB = 8                   # one-hot chunks generated per DVE op

TRACE = False             # test.py sets this for profiling runs
LAST_RESULTS = None       # test.py reads exec_time_ns from here
DEBUG_STAGE = 0           # 4 = stop after conv1, out_ml rows = hc tiles (f32)

_CACHE = {}


def _preprocess(edge_index):
    src = np.asarray(edge_index[0]).astype(np.int64)
    dst = np.asarray(edge_index[1]).astype(np.int64)
    loop = np.arange(N, dtype=np.int64)
    src_all = np.concatenate([src, loop])
    dst_all = np.concatenate([dst, loop])

    deg = np.bincount(dst_all, minlength=N).astype(np.float32)
    dis = (1.0 / np.sqrt(deg)).astype(np.float32)  # deg >= 1 (self loops)

    per_core = []
    cnts = np.zeros((NCORES, NT, 2), np.int64)
    for c in range(NCORES):
        m = (dst_all // SH) == c
        es = src_all[m]
        ed = dst_all[m] - c * SH
        t = ed >> 7
        dl = ed & 127
        g = (es >= LO).astype(np.int64)
        order = np.lexsort((g, t))
        es, t, dl, g = es[order], t[order], dl[order], g[order]
        key = t * 2 + g
        bc = np.bincount(key, minlength=NT * 2)
        cnts[c] = bc.reshape(NT, 2)
        per_core.append((es, t, dl, g, key))

    C = (cnts.max(axis=0) + 127) // 128        # [NT, 2] chunks per (tile, grp)
    KL = int(C[:, 0].sum())                    # total lo chunks
    KH = int(C[:, 1].sum())                    # total hi chunks
    KT = KL + KH

    lo_off = np.concatenate([[0], np.cumsum(C[:, 0])[:-1]])   # chunk offset in lo stream
    hi_off = np.concatenate([[0], np.cumsum(C[:, 1])[:-1]])
    kk_off = np.concatenate([[0], np.cumsum(C.sum(axis=1))[:-1]])  # global chunk index

    core_data = []
    for c in range(NCORES):
        es, t, dl, g, key = per_core[c]
        # rank of each message within its (tile, grp) block
        blk_start = np.concatenate([[0], np.cumsum(cnts[c].reshape(-1))[:-1]])
        rank = np.arange(len(es)) - blk_start[key]
        # position in the per-group padded stream
        stream_chunk_off = np.where(g == 0, lo_off[t], hi_off[t])
        pos = stream_chunk_off * 128 + rank
        slo = np.zeros(KL * 128, np.int16)
        shi = np.zeros(KH * 128, np.int16)
        slo[pos[g == 0]] = es[g == 0].astype(np.int16)
        shi[pos[g == 1]] = (es[g == 1] - LO).astype(np.int16)
        # destT: global chunk order is per tile [lo chunks..., hi chunks...]
        kk = np.where(g == 0, kk_off[t], kk_off[t] + C[t, 0]) + rank // 128
        dest = np.full(KT * 128, 255.0, np.float16)
        dest[kk * 128 + rank % 128] = dl.astype(np.float16)
        idx_lo = np.tile(slo.reshape(-1, 16).T, (8, 1))   # [128, KL*8]
        idx_hi = np.tile(shi.reshape(-1, 16).T, (8, 1))   # [128, KH*8]
        destT = np.ascontiguousarray(dest.reshape(KT, 128).T)  # [128, KT]
        core_data.append((idx_lo, idx_hi, destT))

    # gather batches: [t0, t1) tile ranges
    batches = []
    t0 = 0
    while t0 < NT:
        t1 = min(t0 + TB, NT)
        batches.append((t0, t1))
        t0 = t1
    meta = dict(C=C, KL=KL, KH=KH, KT=KT,
                lo_off=lo_off, hi_off=hi_off, kk_off=kk_off, batches=batches)
    return dis, core_data, meta


def _build_nc(meta):
    import concourse.bass as bass
    import concourse.bacc as bacc
    import concourse.mybir as mybir
    import concourse.tile as tile
    from concourse import library_config

    C = meta["C"]
    KL, KH, KT = meta["KL"], meta["KH"], meta["KT"]
    lo_off, hi_off, kk_off = meta["lo_off"], meta["hi_off"], meta["kk_off"]
    batches = meta["batches"]

    f16 = mybir.dt.float16
    f32 = mybir.dt.float32
    i16 = mybir.dt.int16
    eq = mybir.AluOpType.is_equal
    mult = mybir.AluOpType.mult
    add = mybir.AluOpType.add
    amax = mybir.AluOpType.max

    nc = bacc.Bacc("TRN2", target_bir_lowering=False, debug=False,
                   enable_asserts=True, num_devices=NCORES)

    xTs = nc.dram_tensor("xTs", [128, SH], f16, kind="ExternalInput")
    W1d = nc.dram_tensor("W1d", [128, 128], f16, kind="ExternalInput")
    Wcd = nc.dram_tensor("Wcd", [128, 128], f16, kind="ExternalInput")
    b1rd = nc.dram_tensor("b1rd", [128, 128], f32, kind="ExternalInput")
    bcrd = nc.dram_tensor("bcrd", [128, 128], f32, kind="ExternalInput")
    disT32d = nc.dram_tensor("disT32d", [128, NT], f32, kind="ExternalInput")
    disT16d = nc.dram_tensor("disT16d", [128, NT], f16, kind="ExternalInput")
    iotad = nc.dram_tensor("iotad", [128, OHB * 128], f16, kind="ExternalInput")
    identd = nc.dram_tensor("identd", [128, 128], f16, kind="ExternalInput")
    idxlod = nc.dram_tensor("idxlod", [128, KL * 8], i16, kind="ExternalInput")
    idxhid = nc.dram_tensor("idxhid", [128, KH * 8], i16, kind="ExternalInput")
    destTd = nc.dram_tensor("destTd", [128, KT], f16, kind="ExternalInput")
    out_ml = nc.dram_tensor("out_ml", [SH, 128], f32, kind="ExternalOutput")

    with tile.TileContext(nc) as tc:
        with (
            tc.tile_pool(name="consts", bufs=1) as cpool,
            tc.tile_pool(name="xin", bufs=3) as xpool,
            tc.tile_pool(name="work", bufs=3) as wpool,
            tc.tile_pool(name="oh", bufs=3) as ohpool,
            tc.tile_pool(name="glo", bufs=2) as gpool_lo,
            tc.tile_pool(name="ghi", bufs=2) as gpool_hi,
            tc.tile_pool(name="psA", bufs=2, space="PSUM") as psA,
            tc.tile_pool(name="psB", bufs=2, space="PSUM") as psB,
            tc.tile_pool(name="psT", bufs=2, space="PSUM") as psT,
            tc.tile_pool(name="psH", bufs=2, space="PSUM") as psH,
            tc.tile_pool(name="dram", bufs=1, space="DRAM") as dpool,
        ):
            nc.gpsimd.load_library(library_config.mlp)

            W1sb = cpool.tile([128, 128], f16, tag="W1sb")
            Wcsb = cpool.tile([128, 128], f16, tag="Wcsb")
            b1sb = cpool.tile([128, 128], f32, tag="b1sb")
            bcsb = cpool.tile([128, 128], f32, tag="bcsb")
            dis32sb = cpool.tile([128, NT], f32, tag="dis32sb")
            dis16sb = cpool.tile([128, NT], f16, tag="dis16sb")
            iotasb = cpool.tile([128, OHB * 128], f16, tag="iotasb")
            identsb = cpool.tile([128, 128], f16, tag="identsb")
            idxlosb = cpool.tile([128, KL * 8], i16, tag="idxlosb")
            idxhisb = cpool.tile([128, KH * 8], i16, tag="idxhisb")
            destTsb = cpool.tile([128, KT], f16, tag="destTsb")

            nc.sync.dma_start(W1sb[:], W1d.ap())
            nc.sync.dma_start(Wcsb[:], Wcd.ap())
            nc.sync.dma_start(b1sb[:], b1rd.ap())
            nc.sync.dma_start(bcsb[:], bcrd.ap())
            nc.sync.dma_start(dis32sb[:], disT32d.ap())
            nc.sync.dma_start(dis16sb[:], disT16d.ap())
            nc.sync.dma_start(iotasb[:], iotad.ap())
            nc.sync.dma_start(identsb[:], identd.ap())
            nc.sync.dma_start(idxlosb[:], idxlod.ap())
            nc.sync.dma_start(idxhisb[:], idxhid.ap())
            nc.sync.dma_start(destTsb[:], destTd.ap())

            h0s = dpool.tile([SH, 128], f16, tag="h0s")
            h0f = dpool.tile([NPAD, 128], f16, tag="h0f")
            hcs = dpool.tile([SH, 128], f16, tag="hcs")
            hcf = dpool.tile([NPAD, 128], f16, tag="hcf")

            # ---- Phase A: h0' shard = (x*dis)@W1, rows of my shard ----
            for t in range(NT):
                xt = xpool.tile([128, 128], f16, tag="xt")
                nc.sync.dma_start(xt[:], xTs.ap()[:, t * 128:(t + 1) * 128])
                ps = psA.tile([128, 128], f32, tag="psA")
                nc.tensor.matmul(ps[:], xt[:], W1sb[:], start=True, stop=True)
                ht = xpool.tile([128, 128], f16, tag="ht")
                nc.scalar.copy(ht[:], ps[:])
                nc.sync.dma_start(h0s[t * 128:(t + 1) * 128, :], ht[:])

            nc.gpsimd.collective_compute(
                "AllGather", mybir.AluOpType.bypass,
                replica_groups=[list(range(NCORES))],
                ins=[h0s.opt()], outs=[h0f.opt()],
            )

            def conv_pass(table, is_conv1):
                for (t0, t1) in batches:
                    cl = int(C[t0:t1, 0].sum())
                    ch = int(C[t0:t1, 1].sum())
                    glo = ghi = None
                    if cl:
                        glo = gpool_lo.tile([128, cl, 128], f16, tag="glo")
                        nc.gpsimd.dma_gather(
                            glo[:], table[0:LO, :],
                            idxlosb[:, int(lo_off[t0]) * 8:(int(lo_off[t0]) + cl) * 8],
                            num_idxs=cl * 128, num_idxs_reg=cl * 128,
                            elem_size=128, single_packet=False,
                        )
                    if ch:
                        ghi = gpool_hi.tile([128, ch, 128], f16, tag="ghi")
                        nc.gpsimd.dma_gather(
                            ghi[:], table[LO:NPAD, :],
                            idxhisb[:, int(hi_off[t0]) * 8:(int(hi_off[t0]) + ch) * 8],
                            num_idxs=ch * 128, num_idxs_reg=ch * 128,
                            elem_size=128, single_packet=False,
                        )
                    for t in range(t0, t1):
                        nch = int(C[t, 0] + C[t, 1])
                        kk0 = int(kk_off[t])
                        # one-hot matrices for all chunks of this tile
                        ohs = []
                        j = 0
                        while j < nch:
                            nb = min(OHB, nch - j)
                            oh = ohpool.tile([128, nb, 128], f16, tag="oh")
                            nc.vector.tensor_tensor(
                                oh[:],
                                iotasb[:, 0:nb * 128].rearrange(
                                    "p (c e) -> p c e", e=128),
                                destTsb[:, kk0 + j:kk0 + j + nb].broadcast_to(
                                    [128, nb, 128]),
                                eq,
                            )
                            ohs.append((j, nb, oh))
                            j += nb

                        def oh_at(k):
                            for (jj, nb, oh) in ohs:
                                if jj <= k < jj + nb:
                                    return oh[:, k - jj, :]
                            raise AssertionError

                        ps = psB.tile([128, 128], f32, tag="psB")
                        k = 0
                        for j2 in range(int(C[t, 0])):
                            src = glo[:, int(lo_off[t] - lo_off[t0]) + j2, :]
                            nc.tensor.matmul(ps[:], oh_at(k), src,
                                             start=(k == 0), stop=(k == nch - 1),
                                             skip_group_check=True)
                            k += 1
                        for j2 in range(int(C[t, 1])):
                            src = ghi[:, int(hi_off[t] - hi_off[t0]) + j2, :]
                            nc.tensor.matmul(ps[:], oh_at(k), src,
                                             start=(k == 0), stop=(k == nch - 1),
                                             skip_group_check=True)
                            k += 1

                        if is_conv1:
                            # h = relu(dis*agg + b1); hs = h*dis
                            hti = wpool.tile([128, 128], f16, tag="hti")
                            nc.vector.scalar_tensor_tensor(
                                hti[:], ps[:], dis32sb[:, t:t + 1], b1sb[:],
                                mult, add)
                            hst = wpool.tile([128, 128], f16, tag="hst")
                            nc.vector.tensor_scalar(
                                hst[:], hti[:], 0.0, dis32sb[:, t:t + 1],
                                amax, mult)
                            # hsT = transpose(hs); hc tile = hsT.T @ Wc
                            pst = psT.tile([128, 128], f16, tag="psT")
                            nc.tensor.transpose(pst[:], hst[:], identsb[:])
                            hsT = wpool.tile([128, 128], f16, tag="hsT")
                            nc.scalar.copy(hsT[:], pst[:])
                            psh = psH.tile([128, 128], f32, tag="psH")
                            nc.tensor.matmul(psh[:], hsT[:], Wcsb[:],
                                             start=True, stop=True,
                                             skip_group_check=True)
                            hct = wpool.tile([128, 128], f16, tag="hct")
                            nc.scalar.copy(hct[:], psh[:])
                            nc.sync.dma_start(hcs[t * 128:(t + 1) * 128, :],
                                              hct[:])
                            if DEBUG_STAGE == 4:
                                dbg = wpool.tile([128, 128], f32, tag="dbg")
                                nc.vector.tensor_copy(dbg[:], psh[:])
                                nc.sync.dma_start(
                                    out_ml.ap()[t * 128:(t + 1) * 128, :],
                                    dbg[:])
                        else:
                            ot = wpool.tile([128, 128], f32, tag="ot")
                            if DEBUG_STAGE == 8:
                                nc.vector.tensor_copy(ot[:], ps[:])
                            else:
                                nc.vector.scalar_tensor_tensor(
                                    ot[:], ps[:], dis32sb[:, t:t + 1], bcsb[:],
                                    mult, add)
                            nc.sync.dma_start(out_ml.ap()[t * 128:(t + 1) * 128, :],
                                              ot[:])

            conv_pass(h0f, True)

            if DEBUG_STAGE != 4:
                nc.gpsimd.collective_compute(
                    "AllGather", mybir.AluOpType.bypass,
                    replica_groups=[list(range(NCORES))],
                    ins=[hcs.opt()], outs=[hcf.opt()],
                )

                if DEBUG_STAGE == 7:
                    for t in range(NT):
                        tt = wpool.tile([128, 128], f16, tag="dbg7a")
                        nc.sync.dma_start(tt[:], hcf[t * 128:(t + 1) * 128, :])
                        of = wpool.tile([128, 128], f32, tag="dbg7b")
                        nc.scalar.copy(of[:], tt[:])
                        nc.sync.dma_start(
                            out_ml.ap()[t * 128:(t + 1) * 128, :], of[:])
                else:
                    conv_pass(hcf, False)

    nc.compile()
    return nc


def kernel(x, edge_index, W1, b1, W_mu, b_mu, W_logstd, b_logstd):
    global LAST_RESULTS
    from concourse.bass_utils import run_bass_kernel_spmd

    x = np.asarray(x, dtype=np.float32)
    W1 = np.asarray(W1, dtype=np.float32)
    b1 = np.asarray(b1, dtype=np.float32)
    W_mu = np.asarray(W_mu, dtype=np.float32)
    b_mu = np.asarray(b_mu, dtype=np.float32)
    W_logstd = np.asarray(W_logstd, dtype=np.float32)
    b_logstd = np.asarray(b_logstd, dtype=np.float32)

    key = np.asarray(edge_index).tobytes()[:64] + np.asarray(edge_index).tobytes()[-64:]
    cached = _CACHE.get("k")
    if cached is not None and cached[0] == key:
        _, dis, core_data, meta, nc = cached
    else:
        dis, core_data, meta = _preprocess(edge_index)
        nc = _build_nc(meta)
        _CACHE["k"] = (key, dis, core_data, meta, nc)

    # host-side tensors
    x2T = np.zeros((IN, NPAD), np.float16)
    x2T[:, :N] = (x * dis[:, None]).T.astype(np.float16)
    W1h = W1.astype(np.float16)
    Wch = np.concatenate([W_mu, W_logstd], axis=1).astype(np.float16)
    b1r = np.tile(b1[None, :], (128, 1)).astype(np.float32)
    bcr = np.tile(np.concatenate([b_mu, b_logstd])[None, :], (128, 1)).astype(np.float32)
    disP = np.zeros(NPAD, np.float32)
    disP[:N] = dis
    iota = np.tile(np.arange(128, dtype=np.float16)[None, :], (128, OHB))
    ident = np.eye(128, dtype=np.float16)

    in_maps = []
    for c in range(NCORES):
        idx_lo, idx_hi, destT = core_data[c]
        disSh = disP[c * SH:(c + 1) * SH].reshape(NT, 128).T  # [128, NT]
        in_maps.append({
            "xTs": np.ascontiguousarray(x2T[:, c * SH:(c + 1) * SH]),
            "W1d": W1h, "Wcd": Wch, "b1rd": b1r, "bcrd": bcr,
            "disT32d": np.ascontiguousarray(disSh.astype(np.float32)),
            "disT16d": np.ascontiguousarray(disSh.astype(np.float16)),
            "iotad": np.ascontiguousarray(iota),
            "identd": ident,
            "idxlod": idx_lo, "idxhid": idx_hi, "destTd": destT,
        })

    res = run_bass_kernel_spmd(nc, in_maps, core_ids=list(range(NCORES)),
                               trace=TRACE)
    LAST_RESULTS = res
    full = np.concatenate([res.results[c]["out_ml"] for c in range(NCORES)],
                          axis=0)[:N]
    mu = np.ascontiguousarray(full[:, :OUT])
    logstd = np.ascontiguousarray(full[:, OUT:])
    return (mu, logstd)



# revision 4
# speedup vs baseline: 1.2272x; 1.2272x over previous
"""GCN encoder (3x GCNConv sharing one normalized adjacency) on 8 TRN2 NeuronCores.

Strategy:
  - Fold the symmetric GCN norm  norm(r,c) = dis[r]*dis[c]  into per-node
    scales: pre-scale rows by dis (host for x, epilogue for h), post-scale
    aggregates by dis[c]. Per-edge messages then need no per-edge weights.
  - Shard destination nodes across the 8 cores (6272 nodes/core after
    padding N=50000 -> 50176). Edges live on the core that owns their
    destination (edge-cut partitioning per the sharding hint).
  - Per conv: gather source rows with dma_gather (fp16 rows, 256B), build
    {0,1} one-hot matrices on the vector engine (is_equal vs an iota), and
    scatter-add via TensorE matmuls accumulating in PSUM per 128-dst tile.
  - Node features are republished between convs with AllGather collectives.
  - mu and logstd share one pass: Wc = [W_mu | W_logstd] (both 64 wide).
"""

import numpy as np

N = 50000
E = 800000
IN = 128
HID = 128
OUT = 64
NCORES = 8
SH = 6272                 # nodes per core (padded)
NPAD = SH * NCORES        # 50176
NT = SH // 128            # 49 dst tiles per core
LO = 32768                # rows in the "lo" gather table (int16 limit)
HIR = NPAD - LO           # rows in the "hi" gather table
TB = 6                    # dst tiles per gather batch
OHB = 8                   # one-hot chunks generated per DVE op

TRACE = False             # test.py sets this for profiling runs
LAST_RESULTS = None       # test.py reads exec_time_ns from here
DEBUG_STAGE = 0           # 4 = stop after conv1, out_ml rows = hc tiles (f32)

_CACHE = {}


def _preprocess(edge_index):
    src = np.asarray(edge_index[0]).astype(np.int64)
    dst = np.asarray(edge_index[1]).astype(np.int64)
    loop = np.arange(N, dtype=np.int64)
    src_all = np.concatenate([src, loop])
    dst_all = np.concatenate([dst, loop])

    deg = np.bincount(dst_all, minlength=N).astype(np.float32)
    dis = (1.0 / np.sqrt(deg)).astype(np.float32)  # deg >= 1 (self loops)

    per_core = []
    cnts = np.zeros((NCORES, NT, 2), np.int64)
    for c in range(NCORES):
        m = (dst_all // SH) == c
        es = src_all[m]
        ed = dst_all[m] - c * SH
        t = ed >> 7
        dl = ed & 127
        g = (es >= LO).astype(np.int64)
        order = np.lexsort((g, t))
        es, t, dl, g = es[order], t[order], dl[order], g[order]
        key = t * 2 + g
        bc = np.bincount(key, minlength=NT * 2)
        cnts[c] = bc.reshape(NT, 2)
        per_core.append((es, t, dl, g, key))

    C = (cnts.max(axis=0) + 127) // 128        # [NT, 2] chunks per (tile, grp)
    KL = int(C[:, 0].sum())                    # total lo chunks
    KH = int(C[:, 1].sum())                    # total hi chunks
    KT = KL + KH

    lo_off = np.concatenate([[0], np.cumsum(C[:, 0])[:-1]])   # chunk offset in lo stream
    hi_off = np.concatenate([[0], np.cumsum(C[:, 1])[:-1]])
    kk_off = np.concatenate([[0], np.cumsum(C.sum(axis=1))[:-1]])  # global chunk index

    core_data = []
    for c in range(NCORES):
        es, t, dl, g, key = per_core[c]
        # rank of each message within its (tile, grp) block
        blk_start = np.concatenate([[0], np.cumsum(cnts[c].reshape(-1))[:-1]])
        rank = np.arange(len(es)) - blk_start[key]
        # position in the per-group padded stream
        stream_chunk_off = np.where(g == 0, lo_off[t], hi_off[t])
        pos = stream_chunk_off * 128 + rank
        slo = np.zeros(KL * 128, np.int16)
        shi = np.zeros(KH * 128, np.int16)
        slo[pos[g == 0]] = es[g == 0].astype(np.int16)
        shi[pos[g == 1]] = (es[g == 1] - LO).astype(np.int16)
        # destT: global chunk order is per tile [lo chunks..., hi chunks...]
        kk = np.where(g == 0, kk_off[t], kk_off[t] + C[t, 0]) + rank // 128
        dest = np.full(KT * 128, 255.0, np.float16)
        dest[kk * 128 + rank % 128] = dl.astype(np.float16)
        idx_lo = np.tile(slo.reshape(-1, 16).T, (8, 1))   # [128, KL*8]
        idx_hi = np.tile(shi.reshape(-1, 16).T, (8, 1))   # [128, KH*8]
        destT = np.ascontiguousarray(dest.reshape(KT, 128).T)  # [128, KT]
        core_data.append((idx_lo, idx_hi, destT))

    # gather batches: [t0, t1) tile ranges
    batches = []
    t0 = 0
    while t0 < NT:
        t1 = min(t0 + TB, NT)
        batches.append((t0, t1))
        t0 = t1
    meta = dict(C=C, KL=KL, KH=KH, KT=KT,
                lo_off=lo_off, hi_off=hi_off, kk_off=kk_off, batches=batches)
    return dis, core_data, meta


def _build_nc(meta):
    import concourse.bass as bass
    import concourse.bacc as bacc
    import concourse.mybir as mybir
    import concourse.tile as tile
    from concourse import library_config

    C = meta["C"]
    KL, KH, KT = meta["KL"], meta["KH"], meta["KT"]
    lo_off, hi_off, kk_off = meta["lo_off"], meta["hi_off"], meta["kk_off"]
    batches = meta["batches"]

    f16 = mybir.dt.float16
    f32 = mybir.dt.float32
    i16 = mybir.dt.int16
    eq = mybir.AluOpType.is_equal
    mult = mybir.AluOpType.mult
    add = mybir.AluOpType.add
    amax = mybir.AluOpType.max

    nc = bacc.Bacc("TRN2", target_bir_lowering=False, debug=False,
                   enable_asserts=True, num_devices=NCORES,
                   num_swdge_queues=4)

    xTs = nc.dram_tensor("xTs", [128, SH], f16, kind="ExternalInput")
    W1d = nc.dram_tensor("W1d", [128, 128], f16, kind="ExternalInput")
    Wcd = nc.dram_tensor("Wcd", [128, 128], f16, kind="ExternalInput")
    b1rd = nc.dram_tensor("b1rd", [128, 128], f32, kind="ExternalInput")
    bcrd = nc.dram_tensor("bcrd", [128, 128], f32, kind="ExternalInput")
    disT32d = nc.dram_tensor("disT32d", [128, NT], f32, kind="ExternalInput")
    disT16d = nc.dram_tensor("disT16d", [128, NT], f16, kind="ExternalInput")
    iotad = nc.dram_tensor("iotad", [128, OHB * 128], f16, kind="ExternalInput")
    identd = nc.dram_tensor("identd", [128, 128], f16, kind="ExternalInput")
    idxlod = nc.dram_tensor("idxlod", [128, KL * 8], i16, kind="ExternalInput")
    idxhid = nc.dram_tensor("idxhid", [128, KH * 8], i16, kind="ExternalInput")
    destTd = nc.dram_tensor("destTd", [128, KT], f16, kind="ExternalInput")
    out_ml = nc.dram_tensor("out_ml", [SH, 128], f32, kind="ExternalOutput")

    with tile.TileContext(nc) as tc:
        with (
            tc.tile_pool(name="consts", bufs=1) as cpool,
            tc.tile_pool(name="xin", bufs=3) as xpool,
            tc.tile_pool(name="work", bufs=3) as wpool,
            tc.tile_pool(name="oh", bufs=3) as ohpool,
            tc.tile_pool(name="glo", bufs=4) as gpool_lo,
            tc.tile_pool(name="ghi", bufs=4) as gpool_hi,
            tc.tile_pool(name="psA", bufs=2, space="PSUM") as psA,
            tc.tile_pool(name="psB", bufs=2, space="PSUM") as psB,
            tc.tile_pool(name="psT", bufs=2, space="PSUM") as psT,
            tc.tile_pool(name="psH", bufs=2, space="PSUM") as psH,
            tc.tile_pool(name="dram", bufs=1, space="DRAM") as dpool,
        ):
            nc.gpsimd.load_library(library_config.mlp)

            W1sb = cpool.tile([128, 128], f16, tag="W1sb")
            Wcsb = cpool.tile([128, 128], f16, tag="Wcsb")
            b1sb = cpool.tile([128, 128], f32, tag="b1sb")
            bcsb = cpool.tile([128, 128], f32, tag="bcsb")
            dis32sb = cpool.tile([128, NT], f32, tag="dis32sb")
            dis16sb = cpool.tile([128, NT], f16, tag="dis16sb")
            iotasb = cpool.tile([128, OHB * 128], f16, tag="iotasb")
            identsb = cpool.tile([128, 128], f16, tag="identsb")
            idxlosb = cpool.tile([128, KL * 8], i16, tag="idxlosb")
            idxhisb = cpool.tile([128, KH * 8], i16, tag="idxhisb")
            destTsb = cpool.tile([128, KT], f16, tag="destTsb")

            nc.sync.dma_start(W1sb[:], W1d.ap())
            nc.sync.dma_start(Wcsb[:], Wcd.ap())
            nc.sync.dma_start(b1sb[:], b1rd.ap())
            nc.sync.dma_start(bcsb[:], bcrd.ap())
            nc.sync.dma_start(dis32sb[:], disT32d.ap())
            nc.sync.dma_start(dis16sb[:], disT16d.ap())
            nc.sync.dma_start(iotasb[:], iotad.ap())
            nc.sync.dma_start(identsb[:], identd.ap())
            nc.sync.dma_start(idxlosb[:], idxlod.ap())
            nc.sync.dma_start(idxhisb[:], idxhid.ap())
            nc.sync.dma_start(destTsb[:], destTd.ap())

            h0s = dpool.tile([SH, 128], f16, tag="h0s")
            h0f = dpool.tile([NPAD, 128], f16, tag="h0f")
            hcs = dpool.tile([SH, 128], f16, tag="hcs")
            hcf = dpool.tile([NPAD, 128], f16, tag="hcf")

            # ---- Phase A: h0' shard = (x*dis)@W1, rows of my shard ----
            for t in range(NT):
                xt = xpool.tile([128, 128], f16, tag="xt")
                nc.sync.dma_start(xt[:], xTs.ap()[:, t * 128:(t + 1) * 128])
                ps = psA.tile([128, 128], f32, tag="psA")
                nc.tensor.matmul(ps[:], xt[:], W1sb[:], start=True, stop=True)
                ht = xpool.tile([128, 128], f16, tag="ht")
                nc.scalar.copy(ht[:], ps[:])
                nc.sync.dma_start(h0s[t * 128:(t + 1) * 128, :], ht[:])

            nc.gpsimd.collective_compute(
                "AllGather", mybir.AluOpType.bypass,
                replica_groups=[list(range(NCORES))],
                ins=[h0s.opt()], outs=[h0f.opt()],
            )

            def conv_pass(table, is_conv1):
                qi = 0
                for (t0, t1) in batches:
                    cl = int(C[t0:t1, 0].sum())
                    ch = int(C[t0:t1, 1].sum())
                    glo = ghi = None
                    if cl:
                        glo = gpool_lo.tile([128, cl, 128], f16, tag="glo")
                        nc.gpsimd.dma_gather(
                            glo[:], table[0:LO, :],
                            idxlosb[:, int(lo_off[t0]) * 8:(int(lo_off[t0]) + cl) * 8],
                            num_idxs=cl * 128, num_idxs_reg=cl * 128,
                            elem_size=128, single_packet=False,
                            queue_num=qi % 4,
                        )
                        qi += 1
                    if ch:
                        ghi = gpool_hi.tile([128, ch, 128], f16, tag="ghi")
                        nc.gpsimd.dma_gather(
                            ghi[:], table[LO:NPAD, :],
                            idxhisb[:, int(hi_off[t0]) * 8:(int(hi_off[t0]) + ch) * 8],
                            num_idxs=ch * 128, num_idxs_reg=ch * 128,
                            elem_size=128, single_packet=False,
                            queue_num=qi % 4,
                        )
                        qi += 1
                    for t in range(t0, t1):
                        nch = int(C[t, 0] + C[t, 1])
                        kk0 = int(kk_off[t])
                        # one-hot matrices for all chunks of this tile
                        ohs = []
                        j = 0
                        while j < nch:
                            nb = min(OHB, nch - j)
                            oh = ohpool.tile([128, nb, 128], f16, tag="oh")
                            nc.vector.tensor_tensor(
                                oh[:],
                                iotasb[:, 0:nb * 128].rearrange(
                                    "p (c e) -> p c e", e=128),
                                destTsb[:, kk0 + j:kk0 + j + nb].broadcast_to(
                                    [128, nb, 128]),
                                eq,
                            )
                            ohs.append((j, nb, oh))
                            j += nb

                        def oh_at(k):
                            for (jj, nb, oh) in ohs:
                                if jj <= k < jj + nb:
                                    return oh[:, k - jj, :]
                            raise AssertionError

                        ps = psB.tile([128, 128], f32, tag="psB")
                        k = 0
                        for j2 in range(int(C[t, 0])):
                            src = glo[:, int(lo_off[t] - lo_off[t0]) + j2, :]
                            nc.tensor.matmul(ps[:], oh_at(k), src,
                                             start=(k == 0), stop=(k == nch - 1),
                                             skip_group_check=True)
                            k += 1
                        for j2 in range(int(C[t, 1])):
                            src = ghi[:, int(hi_off[t] - hi_off[t0]) + j2, :]
                            nc.tensor.matmul(ps[:], oh_at(k), src,
                                             start=(k == 0), stop=(k == nch - 1),
                                             skip_group_check=True)
                            k += 1

                        if is_conv1:
                            # h = relu(dis*agg + b1); hs = h*dis
                            hti = wpool.tile([128, 128], f16, tag="hti")
                            nc.vector.scalar_tensor_tensor(
                                hti[:], ps[:], dis32sb[:, t:t + 1], b1sb[:],
                                mult, add)
                            hst = wpool.tile([128, 128], f16, tag="hst")
                            nc.vector.tensor_scalar(
                                hst[:], hti[:], 0.0, dis32sb[:, t:t + 1],
                                amax, mult)
                            # hsT = transpose(hs); hc tile = hsT.T @ Wc
                            pst = psT.tile([128, 128], f16, tag="psT")
                            nc.tensor.transpose(pst[:], hst[:], identsb[:])
                            hsT = wpool.tile([128, 128], f16, tag="hsT")
                            nc.scalar.copy(hsT[:], pst[:])
                            psh = psH.tile([128, 128], f32, tag="psH")
                            nc.tensor.matmul(psh[:], hsT[:], Wcsb[:],
                                             start=True, stop=True,
                                             skip_group_check=True)
                            hct = wpool.tile([128, 128], f16, tag="hct")
                            nc.scalar.copy(hct[:], psh[:])
                            nc.sync.dma_start(hcs[t * 128:(t + 1) * 128, :],
                                              hct[:])
                            if DEBUG_STAGE == 4:
                                dbg = wpool.tile([128, 128], f32, tag="dbg")
                                nc.vector.tensor_copy(dbg[:], psh[:])
                                nc.sync.dma_start(
                                    out_ml.ap()[t * 128:(t + 1) * 128, :],
                                    dbg[:])
                        else:
                            ot = wpool.tile([128, 128], f32, tag="ot")
                            if DEBUG_STAGE == 8:
                                nc.vector.tensor_copy(ot[:], ps[:])
                            else:
                                nc.vector.scalar_tensor_tensor(
                                    ot[:], ps[:], dis32sb[:, t:t + 1], bcsb[:],
                                    mult, add)
                            nc.sync.dma_start(out_ml.ap()[t * 128:(t + 1) * 128, :],
                                              ot[:])

            conv_pass(h0f, True)

            if DEBUG_STAGE != 4:
                nc.gpsimd.collective_compute(
                    "AllGather", mybir.AluOpType.bypass,
                    replica_groups=[list(range(NCORES))],
                    ins=[hcs.opt()], outs=[hcf.opt()],
                )

                if DEBUG_STAGE == 7:
                    for t in range(NT):
                        tt = wpool.tile([128, 128], f16, tag="dbg7a")
                        nc.sync.dma_start(tt[:], hcf[t * 128:(t + 1) * 128, :])
                        of = wpool.tile([128, 128], f32, tag="dbg7b")
                        nc.scalar.copy(of[:], tt[:])
                        nc.sync.dma_start(
                            out_ml.ap()[t * 128:(t + 1) * 128, :], of[:])
                else:
                    conv_pass(hcf, False)

    nc.compile()
    return nc


def kernel(x, edge_index, W1, b1, W_mu, b_mu, W_logstd, b_logstd):
    global LAST_RESULTS
    from concourse.bass_utils import run_bass_kernel_spmd

    x = np.asarray(x, dtype=np.float32)
    W1 = np.asarray(W1, dtype=np.float32)
    b1 = np.asarray(b1, dtype=np.float32)
    W_mu = np.asarray(W_mu, dtype=np.float32)
    b_mu = np.asarray(b_mu, dtype=np.float32)
    W_logstd = np.asarray(W_logstd, dtype=np.float32)
    b_logstd = np.asarray(b_logstd, dtype=np.float32)

    key = np.asarray(edge_index).tobytes()[:64] + np.asarray(edge_index).tobytes()[-64:]
    cached = _CACHE.get("k")
    if cached is not None and cached[0] == key:
        _, dis, core_data, meta, nc = cached
    else:
        dis, core_data, meta = _preprocess(edge_index)
        nc = _build_nc(meta)
        _CACHE["k"] = (key, dis, core_data, meta, nc)

    # host-side tensors
    x2T = np.zeros((IN, NPAD), np.float16)
    x2T[:, :N] = (x * dis[:, None]).T.astype(np.float16)
    W1h = W1.astype(np.float16)
    Wch = np.concatenate([W_mu, W_logstd], axis=1).astype(np.float16)
    b1r = np.tile(b1[None, :], (128, 1)).astype(np.float32)
    bcr = np.tile(np.concatenate([b_mu, b_logstd])[None, :], (128, 1)).astype(np.float32)
    disP = np.zeros(NPAD, np.float32)
    disP[:N] = dis
    iota = np.tile(np.arange(128, dtype=np.float16)[None, :], (128, OHB))
    ident = np.eye(128, dtype=np.float16)

    in_maps = []
    for c in range(NCORES):
        idx_lo, idx_hi, destT = core_data[c]
        disSh = disP[c * SH:(c + 1) * SH].reshape(NT, 128).T  # [128, NT]
        in_maps.append({
            "xTs": np.ascontiguousarray(x2T[:, c * SH:(c + 1) * SH]),
            "W1d": W1h, "Wcd": Wch, "b1rd": b1r, "bcrd": bcr,
            "disT32d": np.ascontiguousarray(disSh.astype(np.float32)),
            "disT16d": np.ascontiguousarray(disSh.astype(np.float16)),
            "iotad": np.ascontiguousarray(iota),
            "identd": ident,
            "idxlod": idx_lo, "idxhid": idx_hi, "destTd": destT,
        })

    res = run_bass_kernel_spmd(nc, in_maps, core_ids=list(range(NCORES)),
                               trace=TRACE)
    LAST_RESULTS = res
    full = np.concatenate([res.results[c]["out_ml"] for c in range(NCORES)],
                          axis=0)[:N]
    mu = np.ascontiguousarray(full[:, :OUT])
    logstd = np.ascontiguousarray(full[:, OUT:])
    return (mu, logstd)



# revision 10
# speedup vs baseline: 1.4393x; 1.1728x over previous
"""GCN encoder (3x GCNConv sharing one normalized adjacency) on 8 TRN2 NeuronCores.

Strategy:
  - Fold the symmetric GCN norm  norm(r,c) = dis[r]*dis[c]  into per-node
    scales: pre-scale rows by dis (host for x, epilogue for h), post-scale
    aggregates by dis[c]. Per-edge messages then need no per-edge weights.
  - Shard destination nodes across the 8 cores (6272 nodes/core after
    padding N=50000 -> 50176). Edges live on the core that owns their
    destination (edge-cut partitioning per the sharding hint).
  - Per conv: gather source rows with dma_gather (fp16 rows, 256B), build
    {0,1} one-hot matrices on the vector engine (is_equal vs an iota), and
    scatter-add via TensorE matmuls accumulating in PSUM per 128-dst tile.
  - Node features are republished between convs with AllGather collectives.
  - mu and logstd share one pass: Wc = [W_mu | W_logstd] (both 64 wide).
"""

import numpy as np

N = 50000
E = 800000
IN = 128
HID = 128
OUT = 64
NCORES = 8
SH = 6272                 # nodes per core (padded)
NPAD = SH * NCORES        # 50176
NT = SH // 128            # 49 dst tiles per core
LO = 32768                # rows in the "lo" gather table (int16 limit)
HIR = NPAD - LO           # rows in the "hi" gather table
TB = 3                    # dst tiles per gather batch
OHB = 8                   # one-hot chunks generated per DVE op

TRACE = False             # test.py sets this for profiling runs
LAST_RESULTS = None       # test.py reads exec_time_ns from here
DEBUG_STAGE = 0           # 4 = stop after conv1, out_ml rows = hc tiles (f32)

_CACHE = {}


def _preprocess(edge_index):
    src = np.asarray(edge_index[0]).astype(np.int64)
    dst = np.asarray(edge_index[1]).astype(np.int64)
    loop = np.arange(N, dtype=np.int64)
    src_all = np.concatenate([src, loop])
    dst_all = np.concatenate([dst, loop])

    deg = np.bincount(dst_all, minlength=N).astype(np.float32)
    dis = (1.0 / np.sqrt(deg)).astype(np.float32)  # deg >= 1 (self loops)

    per_core = []
    cnts = np.zeros((NCORES, NT, 2), np.int64)
    for c in range(NCORES):
        m = (dst_all // SH) == c
        es = src_all[m]
        ed = dst_all[m] - c * SH
        t = ed >> 7
        dl = ed & 127
        g = (es >= LO).astype(np.int64)
        order = np.lexsort((g, t))
        es, t, dl, g = es[order], t[order], dl[order], g[order]
        key = t * 2 + g
        bc = np.bincount(key, minlength=NT * 2)
        cnts[c] = bc.reshape(NT, 2)
        per_core.append((es, t, dl, g, key))

    C = (cnts.max(axis=0) + 127) // 128        # [NT, 2] chunks per (tile, grp)
    KL = int(C[:, 0].sum())                    # total lo chunks
    KH = int(C[:, 1].sum())                    # total hi chunks
    KT = KL + KH

    lo_off = np.concatenate([[0], np.cumsum(C[:, 0])[:-1]])   # chunk offset in lo stream
    hi_off = np.concatenate([[0], np.cumsum(C[:, 1])[:-1]])
    kk_off = np.concatenate([[0], np.cumsum(C.sum(axis=1))[:-1]])  # global chunk index

    core_data = []
    for c in range(NCORES):
        es, t, dl, g, key = per_core[c]
        # rank of each message within its (tile, grp) block
        blk_start = np.concatenate([[0], np.cumsum(cnts[c].reshape(-1))[:-1]])
        rank = np.arange(len(es)) - blk_start[key]
        # position in the per-group padded stream
        stream_chunk_off = np.where(g == 0, lo_off[t], hi_off[t])
        pos = stream_chunk_off * 128 + rank
        slo = np.zeros(KL * 128, np.int16)
        shi = np.zeros(KH * 128, np.int16)
        slo[pos[g == 0]] = es[g == 0].astype(np.int16)
        shi[pos[g == 1]] = (es[g == 1] - LO).astype(np.int16)
        # destT: global chunk order is per tile [lo chunks..., hi chunks...]
        kk = np.where(g == 0, kk_off[t], kk_off[t] + C[t, 0]) + rank // 128
        dest = np.full(KT * 128, 255.0, np.float16)
        dest[kk * 128 + rank % 128] = dl.astype(np.float16)
        idx_lo = np.tile(slo.reshape(-1, 16).T, (8, 1))   # [128, KL*8]
        idx_hi = np.tile(shi.reshape(-1, 16).T, (8, 1))   # [128, KH*8]
        destT = np.ascontiguousarray(dest.reshape(KT, 128).T)  # [128, KT]
        core_data.append((idx_lo, idx_hi, destT))

    # gather batches: [t0, t1) tile ranges
    batches = []
    t0 = 0
    while t0 < NT:
        t1 = min(t0 + TB, NT)
        batches.append((t0, t1))
        t0 = t1
    meta = dict(C=C, KL=KL, KH=KH, KT=KT,
                lo_off=lo_off, hi_off=hi_off, kk_off=kk_off, batches=batches)
    return dis, core_data, meta


def _build_nc(meta):
    import concourse.bass as bass
    import concourse.bacc as bacc
    import concourse.mybir as mybir
    import concourse.tile as tile
    from concourse import library_config

    C = meta["C"]
    KL, KH, KT = meta["KL"], meta["KH"], meta["KT"]
    lo_off, hi_off, kk_off = meta["lo_off"], meta["hi_off"], meta["kk_off"]
    batches = meta["batches"]

    f16 = mybir.dt.float16
    f32 = mybir.dt.float32
    i16 = mybir.dt.int16
    eq = mybir.AluOpType.is_equal
    mult = mybir.AluOpType.mult
    add = mybir.AluOpType.add
    amax = mybir.AluOpType.max

    nc = bacc.Bacc("TRN2", target_bir_lowering=False, debug=False,
                   enable_asserts=True, num_devices=NCORES,
                   num_swdge_queues=4)

    xTs = nc.dram_tensor("xTs", [128, SH], f16, kind="ExternalInput")
    W1d = nc.dram_tensor("W1d", [128, 128], f16, kind="ExternalInput")
    Wcd = nc.dram_tensor("Wcd", [128, 128], f16, kind="ExternalInput")
    b1rd = nc.dram_tensor("b1rd", [128, 128], f32, kind="ExternalInput")
    bcrd = nc.dram_tensor("bcrd", [128, 128], f32, kind="ExternalInput")
    disT32d = nc.dram_tensor("disT32d", [128, NT], f32, kind="ExternalInput")
    disT16d = nc.dram_tensor("disT16d", [128, NT], f16, kind="ExternalInput")
    iotad = nc.dram_tensor("iotad", [128, OHB * 128], f16, kind="ExternalInput")
    identd = nc.dram_tensor("identd", [128, 128], f16, kind="ExternalInput")
    idxlod = nc.dram_tensor("idxlod", [128, KL * 8], i16, kind="ExternalInput")
    idxhid = nc.dram_tensor("idxhid", [128, KH * 8], i16, kind="ExternalInput")
    destTd = nc.dram_tensor("destTd", [128, KT], f16, kind="ExternalInput")
    out_ml = nc.dram_tensor("out_ml", [SH, 128], f32, kind="ExternalOutput")

    with tile.TileContext(nc) as tc:
        with (
            tc.tile_pool(name="consts", bufs=1) as cpool,
            tc.tile_pool(name="xin", bufs=3) as xpool,
            tc.tile_pool(name="work", bufs=3) as wpool,
            tc.tile_pool(name="oh", bufs=3) as ohpool,
            tc.tile_pool(name="glo", bufs=4) as gpool_lo,
            tc.tile_pool(name="ghi", bufs=4) as gpool_hi,
            tc.tile_pool(name="psA", bufs=2, space="PSUM") as psA,
            tc.tile_pool(name="psB", bufs=2, space="PSUM") as psB,
            tc.tile_pool(name="psT", bufs=2, space="PSUM") as psT,
            tc.tile_pool(name="psH", bufs=2, space="PSUM") as psH,
            tc.tile_pool(name="dram", bufs=1, space="DRAM") as dpool,
        ):
            nc.gpsimd.load_library(library_config.mlp)

            W1sb = cpool.tile([128, 128], f16, tag="W1sb")
            Wcsb = cpool.tile([128, 128], f16, tag="Wcsb")
            b1sb = cpool.tile([128, 128], f32, tag="b1sb")
            bcsb = cpool.tile([128, 128], f32, tag="bcsb")
            dis32sb = cpool.tile([128, NT], f32, tag="dis32sb")
            dis16sb = cpool.tile([128, NT], f16, tag="dis16sb")
            iotasb = cpool.tile([128, OHB * 128], f16, tag="iotasb")
            identsb = cpool.tile([128, 128], f16, tag="identsb")
            idxlosb = cpool.tile([128, KL * 8], i16, tag="idxlosb")
            idxhisb = cpool.tile([128, KH * 8], i16, tag="idxhisb")
            destTsb = cpool.tile([128, KT], f16, tag="destTsb")
            zerosb = cpool.tile([128, 1], f32, tag="zerosb")
            nc.vector.memset(zerosb[:], 0.0)

            nc.sync.dma_start(W1sb[:], W1d.ap())
            nc.sync.dma_start(Wcsb[:], Wcd.ap())
            nc.sync.dma_start(b1sb[:], b1rd.ap())
            nc.sync.dma_start(bcsb[:], bcrd.ap())
            nc.sync.dma_start(dis32sb[:], disT32d.ap())
            nc.sync.dma_start(dis16sb[:], disT16d.ap())
            nc.sync.dma_start(iotasb[:], iotad.ap())
            nc.sync.dma_start(identsb[:], identd.ap())
            nc.sync.dma_start(idxlosb[:], idxlod.ap())
            nc.sync.dma_start(idxhisb[:], idxhid.ap())
            nc.sync.dma_start(destTsb[:], destTd.ap())

            h0s = dpool.tile([SH, 128], f16, tag="h0s")
            h0f = dpool.tile([NPAD, 128], f16, tag="h0f")
            hcs = dpool.tile([SH, 128], f16, tag="hcs")
            hcf = dpool.tile([NPAD, 128], f16, tag="hcf")

            # ---- Phase A: h0' shard = (x*dis)@W1, rows of my shard ----
            for t in range(NT):
                xt = xpool.tile([128, 128], f16, tag="xt")
                nc.sync.dma_start(xt[:], xTs.ap()[:, t * 128:(t + 1) * 128])
                ps = psA.tile([128, 128], f32, tag="psA")
                nc.tensor.matmul(ps[:], xt[:], W1sb[:], start=True, stop=True)
                ht = xpool.tile([128, 128], f16, tag="ht")
                nc.scalar.copy(ht[:], ps[:])
                nc.sync.dma_start(h0s[t * 128:(t + 1) * 128, :], ht[:])

            nc.gpsimd.collective_compute(
                "AllGather", mybir.AluOpType.bypass,
                replica_groups=[list(range(NCORES))],
                ins=[h0s.opt()], outs=[h0f.opt()],
            )

            def conv_pass(table, is_conv1):
                qi = 0
                for (t0, t1) in batches:
                    cl = int(C[t0:t1, 0].sum())
                    ch = int(C[t0:t1, 1].sum())
                    glo = ghi = None
                    if cl:
                        glo = gpool_lo.tile([128, cl, 128], f16, tag="glo")
                        nc.gpsimd.dma_gather(
                            glo[:], table[0:LO, :],
                            idxlosb[:, int(lo_off[t0]) * 8:(int(lo_off[t0]) + cl) * 8],
                            num_idxs=cl * 128, num_idxs_reg=cl * 128,
                            elem_size=128, single_packet=False,
                            queue_num=qi % 4,
                        )
                        qi += 1
                    if ch:
                        ghi = gpool_hi.tile([128, ch, 128], f16, tag="ghi")
                        nc.gpsimd.dma_gather(
                            ghi[:], table[LO:NPAD, :],
                            idxhisb[:, int(hi_off[t0]) * 8:(int(hi_off[t0]) + ch) * 8],
                            num_idxs=ch * 128, num_idxs_reg=ch * 128,
                            elem_size=128, single_packet=False,
                            queue_num=qi % 4,
                        )
                        qi += 1
                    for t in range(t0, t1):
                        nch = int(C[t, 0] + C[t, 1])
                        kk0 = int(kk_off[t])
                        # one-hot matrices for all chunks of this tile
                        ohs = []
                        j = 0
                        while j < nch:
                            nb = min(OHB, nch - j)
                            oh = ohpool.tile([128, nb, 128], f16, tag="oh")
                            nc.vector.tensor_tensor(
                                oh[:],
                                iotasb[:, 0:nb * 128].rearrange(
                                    "p (c e) -> p c e", e=128),
                                destTsb[:, kk0 + j:kk0 + j + nb].broadcast_to(
                                    [128, nb, 128]),
                                eq,
                            )
                            ohs.append((j, nb, oh))
                            j += nb

                        def oh_at(k):
                            for (jj, nb, oh) in ohs:
                                if jj <= k < jj + nb:
                                    return oh[:, k - jj, :]
                            raise AssertionError

                        ps = psB.tile([128, 128], f32, tag="psB")
                        k = 0
                        for j2 in range(int(C[t, 0])):
                            src = glo[:, int(lo_off[t] - lo_off[t0]) + j2, :]
                            nc.tensor.matmul(ps[:], oh_at(k), src,
                                             start=(k == 0), stop=(k == nch - 1),
                                             skip_group_check=True)
                            k += 1
                        for j2 in range(int(C[t, 1])):
                            src = ghi[:, int(hi_off[t] - hi_off[t0]) + j2, :]
                            nc.tensor.matmul(ps[:], oh_at(k), src,
                                             start=(k == 0), stop=(k == nch - 1),
                                             skip_group_check=True)
                            k += 1

                        if is_conv1:
                            # h = relu(dis*agg + b1); hs = h*dis
                            # (both as scalar_tensor_tensor: 2-tensor DVE ops
                            # never enter 2-port perf mode, so they don't
                            # block GpSimd SWDGE descriptor generation)
                            hti = wpool.tile([128, 128], f16, tag="hti")
                            nc.vector.scalar_tensor_tensor(
                                hti[:], ps[:], dis32sb[:, t:t + 1], b1sb[:],
                                mult, add)
                            hst = wpool.tile([128, 128], f16, tag="hst")
                            nc.vector.scalar_tensor_tensor(
                                hst[:], hti[:], dis32sb[:, t:t + 1],
                                zerosb[:, 0:1].broadcast_to([128, 128]),
                                mult, amax)
                            # hsT = transpose(hs); hc tile = hsT.T @ Wc
                            pst = psT.tile([128, 128], f16, tag="psT")
                            nc.tensor.transpose(pst[:], hst[:], identsb[:])
                            hsT = wpool.tile([128, 128], f16, tag="hsT")
                            nc.scalar.copy(hsT[:], pst[:])
                            psh = psH.tile([128, 128], f32, tag="psH")
                            nc.tensor.matmul(psh[:], hsT[:], Wcsb[:],
                                             start=True, stop=True,
                                             skip_group_check=True)
                            hct = wpool.tile([128, 128], f16, tag="hct")
                            nc.scalar.copy(hct[:], psh[:])
                            nc.sync.dma_start(hcs[t * 128:(t + 1) * 128, :],
                                              hct[:])
                            if DEBUG_STAGE == 4:
                                dbg = wpool.tile([128, 128], f32, tag="dbg")
                                nc.vector.tensor_copy(dbg[:], psh[:])
                                nc.sync.dma_start(
                                    out_ml.ap()[t * 128:(t + 1) * 128, :],
                                    dbg[:])
                        else:
                            ot = wpool.tile([128, 128], f32, tag="ot")
                            if DEBUG_STAGE == 8:
                                nc.vector.tensor_copy(ot[:], ps[:])
                            else:
                                nc.vector.scalar_tensor_tensor(
                                    ot[:], ps[:], dis32sb[:, t:t + 1], bcsb[:],
                                    mult, add)
                            nc.sync.dma_start(out_ml.ap()[t * 128:(t + 1) * 128, :],
                                              ot[:])

            conv_pass(h0f, True)

            if DEBUG_STAGE != 4:
                nc.gpsimd.collective_compute(
                    "AllGather", mybir.AluOpType.bypass,
                    replica_groups=[list(range(NCORES))],
                    ins=[hcs.opt()], outs=[hcf.opt()],
                )

                if DEBUG_STAGE == 7:
                    for t in range(NT):
                        tt = wpool.tile([128, 128], f16, tag="dbg7a")
                        nc.sync.dma_start(tt[:], hcf[t * 128:(t + 1) * 128, :])
                        of = wpool.tile([128, 128], f32, tag="dbg7b")
                        nc.scalar.copy(of[:], tt[:])
                        nc.sync.dma_start(
                            out_ml.ap()[t * 128:(t + 1) * 128, :], of[:])
                else:
                    conv_pass(hcf, False)

    nc.compile()
    return nc


def kernel(x, edge_index, W1, b1, W_mu, b_mu, W_logstd, b_logstd):
    global LAST_RESULTS
    from concourse.bass_utils import run_bass_kernel_spmd

    x = np.asarray(x, dtype=np.float32)
    W1 = np.asarray(W1, dtype=np.float32)
    b1 = np.asarray(b1, dtype=np.float32)
    W_mu = np.asarray(W_mu, dtype=np.float32)
    b_mu = np.asarray(b_mu, dtype=np.float32)
    W_logstd = np.asarray(W_logstd, dtype=np.float32)
    b_logstd = np.asarray(b_logstd, dtype=np.float32)

    key = np.asarray(edge_index).tobytes()[:64] + np.asarray(edge_index).tobytes()[-64:]
    cached = _CACHE.get("k")
    if cached is not None and cached[0] == key:
        _, dis, core_data, meta, nc = cached
    else:
        dis, core_data, meta = _preprocess(edge_index)
        nc = _build_nc(meta)
        _CACHE["k"] = (key, dis, core_data, meta, nc)

    # host-side tensors
    x2T = np.zeros((IN, NPAD), np.float16)
    x2T[:, :N] = (x * dis[:, None]).T.astype(np.float16)
    W1h = W1.astype(np.float16)
    Wch = np.concatenate([W_mu, W_logstd], axis=1).astype(np.float16)
    b1r = np.tile(b1[None, :], (128, 1)).astype(np.float32)
    bcr = np.tile(np.concatenate([b_mu, b_logstd])[None, :], (128, 1)).astype(np.float32)
    disP = np.zeros(NPAD, np.float32)
    disP[:N] = dis
    iota = np.tile(np.arange(128, dtype=np.float16)[None, :], (128, OHB))
    ident = np.eye(128, dtype=np.float16)

    in_maps = []
    for c in range(NCORES):
        idx_lo, idx_hi, destT = core_data[c]
        disSh = disP[c * SH:(c + 1) * SH].reshape(NT, 128).T  # [128, NT]
        in_maps.append({
            "xTs": np.ascontiguousarray(x2T[:, c * SH:(c + 1) * SH]),
            "W1d": W1h, "Wcd": Wch, "b1rd": b1r, "bcrd": bcr,
            "disT32d": np.ascontiguousarray(disSh.astype(np.float32)),
            "disT16d": np.ascontiguousarray(disSh.astype(np.float16)),
            "iotad": np.ascontiguousarray(iota),
            "identd": ident,
            "idxlod": idx_lo, "idxhid": idx_hi, "destTd": destT,
        })

    res = run_bass_kernel_spmd(nc, in_maps, core_ids=list(range(NCORES)),
                               trace=TRACE)
    LAST_RESULTS = res
    full = np.concatenate([res.results[c]["out_ml"] for c in range(NCORES)],
                          axis=0)[:N]
    mu = np.ascontiguousarray(full[:, :OUT])
    logstd = np.ascontiguousarray(full[:, OUT:])
    return (mu, logstd)



# revision 14
# speedup vs baseline: 1.8626x; 1.2941x over previous
"""GCN encoder (3x GCNConv sharing one normalized adjacency) on 8 TRN2 NeuronCores.

v2 strategy:
  - Fold the symmetric GCN norm into per-node scales (as v1): pre-scale x by
    dis on host, post-scale aggregates by dis[dst] in epilogues.
  - Phase A computes the FULL h0 = (x*dis)@W1 on every core (redundant dense
    GEMM, ~40us) -- kills the first AllGather entirely.
  - Shard destination nodes across 8 cores; per-edge source rows fetched with
    dma_gather spread over 4 SWDGE queues (desc-gen parallelism on 4 Q7 core
    pairs), scatter-add via one-hot TensorE matmuls in PSUM.
  - conv1 aggregates hs = dis*relu(dis*agg+b1) WITHOUT applying Wc (GCN is
    linear after aggregation: agg2@Wc == (A@hs)@Wc), so conv2 gathers hs.
  - Node rows live in chunk-major permuted order: chunk c holds tiles
    [c*CT,(c+1)*CT) of every rank contiguously, so the hs republish runs as
    NCH chunked AllGathers overlapped with conv1's tail.
  - conv2 accumulates TRANSPOSED (lhsT=gathered, rhs=one-hot -> psum[h,d]),
    then applies Wc per dst tile: out[d,:] = (aggT)^T@Wc via lhsT=aggT.
  - DVE ops restricted to tensor_tensor/scalar_tensor_tensor (1-port) so they
    never block GpSimd SWDGE descriptor generation.
"""

import numpy as np

N = 50000
E = 800000
IN = 128
HID = 128
OUT = 64
NCORES = 8
SH = 6272                 # nodes per core (padded)
NPAD = SH * NCORES        # 50176
NT = SH // 128            # 49 dst tiles per core
GT = NPAD // 128          # 392 global node tiles
CT = 7                    # dst tiles per AllGather chunk
NCH = NT // CT            # 7 chunks
LO = 32768                # rows in the "lo" gather table (int16 limit)
TB = 3                    # dst tiles per gather batch
OHB = 8                   # one-hot chunks generated per DVE op
NQ = 4                    # SWDGE queues

TRACE = False             # test.py sets this for profiling runs
LAST_RESULTS = None       # test.py reads exec_time_ns from here

_CACHE = {}

# permuted position of natural global tile g = r*NT + t:
#   chunk c = t//CT ; p = c*(NCORES*CT) + r*CT + (t%CT)
_PT = np.empty(GT, np.int64)
for _g in range(GT):
    _r, _t = divmod(_g, NT)
    _c, _tt = divmod(_t, CT)
    _PT[_g] = (_c * NCORES + _r) * CT + _tt


def _pnode(n):
    """natural node id -> permuted row index (vectorized)."""
    return _PT[n >> 7] * 128 + (n & 127)


def _preprocess(edge_index):
    src = np.asarray(edge_index[0]).astype(np.int64)
    dst = np.asarray(edge_index[1]).astype(np.int64)
    loop = np.arange(N, dtype=np.int64)
    src_all = np.concatenate([src, loop])
    dst_all = np.concatenate([dst, loop])

    deg = np.bincount(dst_all, minlength=N).astype(np.float32)
    dis = (1.0 / np.sqrt(deg)).astype(np.float32)  # deg >= 1 (self loops)

    srcp_all = _pnode(src_all)                     # permuted source rows

    per_core = []
    cnts = np.zeros((NCORES, NT, 2), np.int64)
    for c in range(NCORES):
        m = (dst_all // SH) == c
        es = srcp_all[m]
        ed = dst_all[m] - c * SH
        t = ed >> 7
        dl = ed & 127
        g = (es >= LO).astype(np.int64)
        order = np.lexsort((g, t))
        es, t, dl, g = es[order], t[order], dl[order], g[order]
        key = t * 2 + g
        bc = np.bincount(key, minlength=NT * 2)
        cnts[c] = bc.reshape(NT, 2)
        per_core.append((es, t, dl, g, key))

    C = (cnts.max(axis=0) + 127) // 128        # [NT, 2] chunks per (tile, grp)
    KL = int(C[:, 0].sum())                    # total lo chunks
    KH = int(C[:, 1].sum())                    # total hi chunks
    KT = KL + KH

    lo_off = np.concatenate([[0], np.cumsum(C[:, 0])[:-1]])
    hi_off = np.concatenate([[0], np.cumsum(C[:, 1])[:-1]])
    kk_off = np.concatenate([[0], np.cumsum(C.sum(axis=1))[:-1]])

    core_data = []
    for c in range(NCORES):
        es, t, dl, g, key = per_core[c]
        blk_start = np.concatenate([[0], np.cumsum(cnts[c].reshape(-1))[:-1]])
        rank = np.arange(len(es)) - blk_start[key]
        stream_chunk_off = np.where(g == 0, lo_off[t], hi_off[t])
        pos = stream_chunk_off * 128 + rank
        slo = np.zeros(KL * 128, np.int16)
        shi = np.zeros(KH * 128, np.int16)
        slo[pos[g == 0]] = es[g == 0].astype(np.int16)
        shi[pos[g == 1]] = (es[g == 1] - LO).astype(np.int16)
        kk = np.where(g == 0, kk_off[t], kk_off[t] + C[t, 0]) + rank // 128
        dest = np.full(KT * 128, 255.0, np.float16)
        dest[kk * 128 + rank % 128] = dl.astype(np.float16)
        idx_lo = np.tile(slo.reshape(-1, 16).T, (8, 1))   # [128, KL*8]
        idx_hi = np.tile(shi.reshape(-1, 16).T, (8, 1))   # [128, KH*8]
        destT = np.ascontiguousarray(dest.reshape(KT, 128).T)  # [128, KT]
        core_data.append((idx_lo, idx_hi, destT))

    batches = []
    t0 = 0
    while t0 < NT:
        t1 = min(t0 + TB, NT)
        batches.append((t0, t1))
        t0 = t1
    meta = dict(C=C, KL=KL, KH=KH, KT=KT,
                lo_off=lo_off, hi_off=hi_off, kk_off=kk_off, batches=batches)
    return dis, core_data, meta


def _build_nc(meta):
    import concourse.bass as bass
    import concourse.bacc as bacc
    import concourse.mybir as mybir
    import concourse.tile as tile
    from concourse import library_config

    C = meta["C"]
    KL, KH, KT = meta["KL"], meta["KH"], meta["KT"]
    lo_off, hi_off, kk_off = meta["lo_off"], meta["hi_off"], meta["kk_off"]
    batches = meta["batches"]

    f16 = mybir.dt.float16
    f32 = mybir.dt.float32
    i16 = mybir.dt.int16
    eq = mybir.AluOpType.is_equal
    mult = mybir.AluOpType.mult
    add = mybir.AluOpType.add
    amax = mybir.AluOpType.max

    nc = bacc.Bacc("TRN2", target_bir_lowering=False, debug=False,
                   enable_asserts=True, num_devices=NCORES,
                   num_swdge_queues=NQ)

    xTs = nc.dram_tensor("xTs", [128, NPAD], f16, kind="ExternalInput")
    W1d = nc.dram_tensor("W1d", [128, 128], f16, kind="ExternalInput")
    Wcd = nc.dram_tensor("Wcd", [128, 128], f16, kind="ExternalInput")
    b1rd = nc.dram_tensor("b1rd", [128, 128], f32, kind="ExternalInput")
    bcrd = nc.dram_tensor("bcrd", [128, 128], f32, kind="ExternalInput")
    disT32d = nc.dram_tensor("disT32d", [128, NT], f32, kind="ExternalInput")
    iotad = nc.dram_tensor("iotad", [128, OHB * 128], f16, kind="ExternalInput")
    idxlod = nc.dram_tensor("idxlod", [128, KL * 8], i16, kind="ExternalInput")
    idxhid = nc.dram_tensor("idxhid", [128, KH * 8], i16, kind="ExternalInput")
    destTd = nc.dram_tensor("destTd", [128, KT], f16, kind="ExternalInput")
    out_ml = nc.dram_tensor("out_ml", [SH, 128], f32, kind="ExternalOutput")

    with tile.TileContext(nc) as tc:
        with (
            tc.tile_pool(name="consts", bufs=1) as cpool,
            tc.tile_pool(name="xin", bufs=3) as xpool,
            tc.tile_pool(name="hbuf", bufs=3) as hpool,
            tc.tile_pool(name="work", bufs=4) as wpool,
            tc.tile_pool(name="oh", bufs=4) as ohpool,
            tc.tile_pool(name="glo", bufs=4) as gpool_lo,
            tc.tile_pool(name="ghi", bufs=4) as gpool_hi,
            tc.tile_pool(name="psA", bufs=2, space="PSUM") as psA,
            tc.tile_pool(name="psB", bufs=2, space="PSUM") as psB,
            tc.tile_pool(name="psO", bufs=2, space="PSUM") as psO,
            tc.tile_pool(name="dram", bufs=1, space="DRAM") as dpool,
        ):
            nc.gpsimd.load_library(library_config.mlp)

            W1sb = cpool.tile([128, 128], f16, tag="W1sb")
            Wcsb = cpool.tile([128, 128], f16, tag="Wcsb")
            b1sb = cpool.tile([128, 128], f32, tag="b1sb")
            bcsb = cpool.tile([128, 128], f32, tag="bcsb")
            dis32sb = cpool.tile([128, NT], f32, tag="dis32sb")
            iotasb = cpool.tile([128, OHB * 128], f16, tag="iotasb")
            idxlosb = cpool.tile([128, KL * 8], i16, tag="idxlosb")
            idxhisb = cpool.tile([128, KH * 8], i16, tag="idxhisb")
            destTsb = cpool.tile([128, KT], f16, tag="destTsb")
            zerosb = cpool.tile([128, 1], f32, tag="zerosb")
            nc.vector.memset(zerosb[:], 0.0)

            nc.sync.dma_start(W1sb[:], W1d.ap())
            nc.sync.dma_start(Wcsb[:], Wcd.ap())
            nc.sync.dma_start(b1sb[:], b1rd.ap())
            nc.sync.dma_start(bcsb[:], bcrd.ap())
            nc.sync.dma_start(dis32sb[:], disT32d.ap())
            nc.sync.dma_start(iotasb[:], iotad.ap())
            nc.sync.dma_start(idxlosb[:], idxlod.ap())
            nc.sync.dma_start(idxhisb[:], idxhid.ap())
            nc.sync.dma_start(destTsb[:], destTd.ap())

            h0f = dpool.tile([NPAD, 128], f16, tag="h0f")
            hsf = dpool.tile([NPAD, 128], f16, tag="hsf")
            hss = [dpool.tile([CT * 128, 128], f16, tag=f"hss{c}",
                              name=f"hss{c}")
                   for c in range(NCH)]

            # ---- Phase A: full h0 = (x*dis)@W1, all NPAD rows, locally ----
            XB = 8                      # node tiles per bulk DMA
            for g0 in range(0, GT, XB):
                xb = xpool.tile([128, XB * 128], f16, tag="xb")
                nc.sync.dma_start(xb[:], xTs.ap()[:, g0 * 128:(g0 + XB) * 128])
                hb = hpool.tile([128, XB * 128], f16, tag="hb")
                for j in range(XB):
                    ps = psA.tile([128, 128], f32, tag="psA")
                    nc.tensor.matmul(ps[:], xb[:, j * 128:(j + 1) * 128],
                                     W1sb[:], start=True, stop=True)
                    nc.scalar.copy(hb[:, j * 128:(j + 1) * 128], ps[:])
                nc.sync.dma_start(
                    h0f[g0 * 128:(g0 + XB) * 128, :].rearrange(
                        "(j p) q -> p j q", j=XB),
                    hb[:].rearrange("p (j q) -> p j q", q=128))

            def conv_pass(table, is_conv1):
                qi = 0
                for (t0, t1) in batches:
                    cl = int(C[t0:t1, 0].sum())
                    ch = int(C[t0:t1, 1].sum())
                    glo = ghi = None
                    if cl:
                        glo = gpool_lo.tile([128, cl, 128], f16, tag="glo")
                        nc.gpsimd.dma_gather(
                            glo[:], table[0:LO, :],
                            idxlosb[:, int(lo_off[t0]) * 8:(int(lo_off[t0]) + cl) * 8],
                            num_idxs=cl * 128, num_idxs_reg=cl * 128,
                            elem_size=128, single_packet=False,
                            queue_num=qi % NQ,
                        )
                        qi += 1
                    if ch:
                        ghi = gpool_hi.tile([128, ch, 128], f16, tag="ghi")
                        nc.gpsimd.dma_gather(
                            ghi[:], table[LO:NPAD, :],
                            idxhisb[:, int(hi_off[t0]) * 8:(int(hi_off[t0]) + ch) * 8],
                            num_idxs=ch * 128, num_idxs_reg=ch * 128,
                            elem_size=128, single_packet=False,
                            queue_num=qi % NQ,
                        )
                        qi += 1
                    for t in range(t0, t1):
                        nch = int(C[t, 0] + C[t, 1])
                        kk0 = int(kk_off[t])
                        ohs = []
                        j = 0
                        while j < nch:
                            nb = min(OHB, nch - j)
                            oh = ohpool.tile([128, nb, 128], f16, tag="oh")
                            nc.vector.tensor_tensor(
                                oh[:],
                                iotasb[:, 0:nb * 128].rearrange(
                                    "p (c e) -> p c e", e=128),
                                destTsb[:, kk0 + j:kk0 + j + nb].broadcast_to(
                                    [128, nb, 128]),
                                eq,
                            )
                            ohs.append((j, nb, oh))
                            j += nb

                        def oh_at(k):
                            for (jj, nb, oh) in ohs:
                                if jj <= k < jj + nb:
                                    return oh[:, k - jj, :]
                            raise AssertionError

                        def src_at(k):
                            if k < int(C[t, 0]):
                                return glo[:, int(lo_off[t] - lo_off[t0]) + k, :]
                            return ghi[:, int(hi_off[t] - hi_off[t0])
                                       + (k - int(C[t, 0])), :]

                        ps = psB.tile([128, 128], f32, tag="psB")
                        for k in range(nch):
                            if is_conv1:
                                # psum[d, h] += oh^T @ src
                                nc.tensor.matmul(ps[:], oh_at(k), src_at(k),
                                                 start=(k == 0),
                                                 stop=(k == nch - 1),
                                                 skip_group_check=True)
                            else:
                                # psum[h, d] += src^T @ oh  (transposed agg)
                                nc.tensor.matmul(ps[:], src_at(k), oh_at(k),
                                                 start=(k == 0),
                                                 stop=(k == nch - 1),
                                                 skip_group_check=True)

                        if is_conv1:
                            # hs = dis * relu(dis*agg + b1)
                            hti = wpool.tile([128, 128], f16, tag="hti")
                            nc.vector.scalar_tensor_tensor(
                                hti[:], ps[:], dis32sb[:, t:t + 1], b1sb[:],
                                mult, add)
                            hst = wpool.tile([128, 128], f16, tag="hst")
                            nc.vector.scalar_tensor_tensor(
                                hst[:], hti[:], dis32sb[:, t:t + 1],
                                zerosb[:, 0:1].broadcast_to([128, 128]),
                                mult, amax)
                            c, tt = divmod(t, CT)
                            nc.sync.dma_start(
                                hss[c][tt * 128:(tt + 1) * 128, :], hst[:])
                            if tt == CT - 1:
                                nc.gpsimd.collective_compute(
                                    "AllGather", mybir.AluOpType.bypass,
                                    replica_groups=[list(range(NCORES))],
                                    ins=[hss[c].opt()],
                                    outs=[hsf[c * CT * 128 * NCORES:
                                              (c + 1) * CT * 128 * NCORES, :]],
                                )
                        else:
                            # out[d, :] = dis*(aggT^T @ Wc) + bc
                            sT = wpool.tile([128, 128], f16, tag="sT")
                            nc.scalar.copy(sT[:], ps[:])
                            po = psO.tile([128, 128], f32, tag="psO")
                            nc.tensor.matmul(po[:], sT[:], Wcsb[:],
                                             start=True, stop=True,
                                             skip_group_check=True)
                            ot = wpool.tile([128, 128], f32, tag="ot")
                            nc.vector.scalar_tensor_tensor(
                                ot[:], po[:], dis32sb[:, t:t + 1], bcsb[:],
                                mult, add)
                            nc.sync.dma_start(
                                out_ml.ap()[t * 128:(t + 1) * 128, :], ot[:])

            conv_pass(h0f, True)
            conv_pass(hsf, False)

    nc.compile()
    return nc


def kernel(x, edge_index, W1, b1, W_mu, b_mu, W_logstd, b_logstd):
    global LAST_RESULTS
    from concourse.bass_utils import run_bass_kernel_spmd

    x = np.asarray(x, dtype=np.float32)
    W1 = np.asarray(W1, dtype=np.float32)
    b1 = np.asarray(b1, dtype=np.float32)
    W_mu = np.asarray(W_mu, dtype=np.float32)
    b_mu = np.asarray(b_mu, dtype=np.float32)
    W_logstd = np.asarray(W_logstd, dtype=np.float32)
    b_logstd = np.asarray(b_logstd, dtype=np.float32)

    key = np.asarray(edge_index).tobytes()[:64] + np.asarray(edge_index).tobytes()[-64:]
    cached = _CACHE.get("k")
    if cached is not None and cached[0] == key:
        _, dis, core_data, meta, nc = cached
    else:
        dis, core_data, meta = _preprocess(edge_index)
        nc = _build_nc(meta)
        _CACHE["k"] = (key, dis, core_data, meta, nc)

    # host-side tensors: x2T columns in permuted row order
    xs = (x * dis[:, None]).astype(np.float32)
    x2T = np.zeros((IN, NPAD), np.float16)
    x2T[:, _pnode(np.arange(N))] = xs.T.astype(np.float16)
    W1h = W1.astype(np.float16)
    Wch = np.concatenate([W_mu, W_logstd], axis=1).astype(np.float16)
    b1r = np.tile(b1[None, :], (128, 1)).astype(np.float32)
    bcr = np.tile(np.concatenate([b_mu, b_logstd])[None, :], (128, 1)).astype(np.float32)
    disP = np.zeros(NPAD, np.float32)
    disP[:N] = dis
    iota = np.tile(np.arange(128, dtype=np.float16)[None, :], (128, OHB))

    in_maps = []
    for c in range(NCORES):
        idx_lo, idx_hi, destT = core_data[c]
        disSh = disP[c * SH:(c + 1) * SH].reshape(NT, 128).T  # [128, NT]
        in_maps.append({
            "xTs": x2T,
            "W1d": W1h, "Wcd": Wch, "b1rd": b1r, "bcrd": bcr,
            "disT32d": np.ascontiguousarray(disSh.astype(np.float32)),
            "iotad": np.ascontiguousarray(iota),
            "idxlod": idx_lo, "idxhid": idx_hi, "destTd": destT,
        })

    res = run_bass_kernel_spmd(nc, in_maps, core_ids=list(range(NCORES)),
                               trace=TRACE)
    LAST_RESULTS = res
    full = np.concatenate([res.results[c]["out_ml"] for c in range(NCORES)],
                          axis=0)[:N]
    mu = np.ascontiguousarray(full[:, :OUT])
    logstd = np.ascontiguousarray(full[:, OUT:])
    return (mu, logstd)
